# revision 1
# baseline (speedup 1.0000x reference)
# Trainium2 Bass kernel for nn_Connection_geognn_17076789969601.
#
# Math (per sample row of input_ [N, 128], x = row[:64], v = row[64:]):
#   h  = tanh(W1 @ x + b1)                  # [128]
#   Wm = tanh(W2 @ h + b2).reshape(64, 4)   # [64, 4]
#   u  = v @ Wm ;  H = sum(u^2)
#   output = [dH/dx, -dH/dv]
#
# Backward (per sample), with T = tanh(A2) in W2-row-permuted order so that
# column j of Wm occupies rows [64j, 64j+64):
#   dv_out = -2 Wm u
#   dA2    = 2 (v x u) * (1 - T^2) = Q - M,  Q = 2u*v,  M = 2u*v*T^2
#   dh     = W2r^T dA2 ;  dA1 = (1-h^2)*dh ;  dx = W1^T dA1
#
# Device layout: feature-major streams [feat(part), samples(free)], B=512
# samples per macro tile.  Each PSUM tag owns one bank (a1, a2a, a2b, ra, rb,
# dh1 single-buffered + outq double-buffered = 8 banks), which keeps the Tile
# pipeline free of PSUM WAR stalls - measured much faster than any wider-tile
# variant.  Input/output are transposed ON HOST so all DMA is contiguous.
# u's block-sum+broadcast (mblk) and the dv pair-sum (msum) are constant mask
# matmuls; dA2 is never materialized - Q and M feed sign-folded accumulating
# matmuls for dh.
#
# Engine placement per tile: Act {tanh x3, R-copy x2, h1sq}; DVE {P x2, S x2,
# Q x2, dA1-STT, out-copy}; GpSimd {M x2}; PE {12 matmuls}.
#
# Sharding: pure data parallel over 8 NeuronCores, batch 262144 -> 8 x 32768,
# weights replicated.

import sys

sys.path.insert(0, "/opt/trn_rl_repo")

import numpy as np
import ml_dtypes

import concourse.bass as bass
import concourse.bacc as bacc
import concourse.tile as tile
import concourse.mybir as mybir
from concourse.bass_utils import run_bass_kernel_spmd

F32 = mybir.dt.float32
BF16 = mybir.dt.bfloat16
AF = mybir.ActivationFunctionType
ALU = mybir.AluOpType

D = 64
RANK = 4
N_TOTAL = 262144
N_CORES = 8
N_ROWS = N_TOTAL // N_CORES  # 32768 per core
B = 512                      # samples per macro tile


def build_program(n_rows=N_ROWS, b=B):
    nt = n_rows // b
    nc = bacc.Bacc()

    inp = nc.declare_dram_parameter("inp", [128, n_rows], BF16, isOutput=False)
    w1t = nc.declare_dram_parameter("w1t", [64, 128], BF16, isOutput=False)
    w2ta = nc.declare_dram_parameter("w2ta", [128, 128], BF16, isOutput=False)
    w2tb = nc.declare_dram_parameter("w2tb", [128, 128], BF16, isOutput=False)
    w2pa = nc.declare_dram_parameter("w2pa", [128, 128], BF16, isOutput=False)
    w2pb = nc.declare_dram_parameter("w2pb", [128, 128], BF16, isOutput=False)
    w2ma = nc.declare_dram_parameter("w2ma", [128, 128], BF16, isOutput=False)
    w2mb = nc.declare_dram_parameter("w2mb", [128, 128], BF16, isOutput=False)
    w1n = nc.declare_dram_parameter("w1n", [128, 64], BF16, isOutput=False)
    mblk = nc.declare_dram_parameter("mblk", [128, 128], BF16, isOutput=False)
    msum = nc.declare_dram_parameter("msum", [128, 64], BF16, isOutput=False)
    b1p = nc.declare_dram_parameter("b1", [128, 1], F32, isOutput=False)
    b2ap = nc.declare_dram_parameter("b2a", [128, 1], F32, isOutput=False)
    b2bp = nc.declare_dram_parameter("b2b", [128, 1], F32, isOutput=False)
    outp = nc.declare_dram_parameter("out", [128, n_rows], BF16, isOutput=True)

    with tile.TileContext(nc) as tc:
        with (
            tc.tile_pool(name="const", bufs=1) as cp,
            tc.tile_pool(name="sb", bufs=4) as sb,
            tc.tile_pool(name="ps", bufs=1, space="PSUM") as ps,
        ):
            c_w1t = cp.tile([64, 128], BF16, tag="w1t")
            c_w2ta = cp.tile([128, 128], BF16, tag="w2ta")
            c_w2tb = cp.tile([128, 128], BF16, tag="w2tb")
            c_w2pa = cp.tile([128, 128], BF16, tag="w2pa")
            c_w2pb = cp.tile([128, 128], BF16, tag="w2pb")
            c_w2ma = cp.tile([128, 128], BF16, tag="w2ma")
            c_w2mb = cp.tile([128, 128], BF16, tag="w2mb")
            c_w1n = cp.tile([128, 64], BF16, tag="w1n")
            c_mblk = cp.tile([128, 128], BF16, tag="mblk")
            c_msum = cp.tile([128, 64], BF16, tag="msum")
            c_b1 = cp.tile([128, 1], F32, tag="b1")
            c_b2a = cp.tile([128, 1], F32, tag="b2a")
            c_b2b = cp.tile([128, 1], F32, tag="b2b")
            for t_, p_ in (
                (c_w1t, w1t), (c_w2ta, w2ta), (c_w2tb, w2tb),
                (c_w2pa, w2pa), (c_w2pb, w2pb), (c_w2ma, w2ma),
                (c_w2mb, w2mb), (c_w1n, w1n), (c_mblk, mblk),
                (c_msum, msum), (c_b1, b1p), (c_b2a, b2ap), (c_b2b, b2bp),
            ):
                nc.sync.dma_start(t_[:], p_[:])

            for t in range(nt):
                # ---- contiguous input load + v replication (DMA only) ----
                tint = sb.tile([128, b], BF16, tag="INT")   # [x^T; v^T]
                nc.sync.dma_start(tint[:], inp[:, bass.ts(t, b)])
                vtile = sb.tile([128, b], BF16, tag="VT")   # [v^T; v^T]
                nc.sync.dma_start(vtile[0:64, :], tint[64:128, :])
                nc.sync.dma_start(vtile[64:128, :], tint[64:128, :])

                # ---- forward layer 1 ----
                a1 = ps.tile([128, b], F32, tag="a1")
                nc.tensor.matmul(a1[:], c_w1t[:], tint[0:64, :],
                                 start=True, stop=True)
                h1 = sb.tile([128, b], BF16, tag="H1")
                nc.scalar.activation(h1[:], a1[:], AF.Tanh, bias=c_b1[:, 0:1])

                # ---- forward layer 2 (W2 rows permuted; two 128-row halves) --
                a2a = ps.tile([128, b], F32, tag="a2a")
                a2b = ps.tile([128, b], F32, tag="a2b")
                nc.tensor.matmul(a2a[:], c_w2ta[:], h1[:], start=True, stop=True)
                nc.tensor.matmul(a2b[:], c_w2tb[:], h1[:], start=True, stop=True)
                t2a = sb.tile([128, b], BF16, tag="T2a")
                t2b = sb.tile([128, b], BF16, tag="T2b")
                nc.scalar.activation(t2a[:], a2a[:], AF.Tanh, bias=c_b2a[:, 0:1])
                nc.scalar.activation(t2b[:], a2b[:], AF.Tanh, bias=c_b2b[:, 0:1])

                # ---- P = T2*vrep ; R = mblk @ P = 2u broadcast (PSUM) ----
                pa = sb.tile([128, b], BF16, tag="Pa")
                pb = sb.tile([128, b], BF16, tag="Pb")
                nc.vector.tensor_mul(pa[:], t2a[:], vtile[:])
                nc.vector.tensor_mul(pb[:], t2b[:], vtile[:])
                ra = ps.tile([128, b], F32, tag="ra")
                rb = ps.tile([128, b], F32, tag="rb")
                nc.tensor.matmul(ra[:], c_mblk[:], pa[:], start=True, stop=True)
                nc.tensor.matmul(rb[:], c_mblk[:], pb[:], start=True, stop=True)
                rca = sb.tile([128, b], BF16, tag="RCa")
                rcb = sb.tile([128, b], BF16, tag="RCb")
                nc.scalar.copy(rca[:], ra[:])
                nc.scalar.copy(rcb[:], rb[:])

                # ---- S = Rc*T2 -> dv (sign folded into msum) ----
                sa = sb.tile([128, b], BF16, tag="Sa")
                sbt = sb.tile([128, b], BF16, tag="Sb")
                nc.vector.tensor_mul(sa[:], rca[:], t2a[:])
                nc.vector.tensor_mul(sbt[:], rcb[:], t2b[:])
                outq = ps.tile([128, b], F32, tag="outq", bufs=2)
                nc.tensor.matmul(outq[64:128, :], c_msum[:], sa[:],
                                 start=True, stop=False)
                nc.tensor.matmul(outq[64:128, :], c_msum[:], sbt[:],
                                 start=False, stop=True)

                # ---- Q = Rc*vrep (DVE), M = S*P (GpSimd) ----
                qa = sb.tile([128, b], BF16, tag="Qa")
                qb = sb.tile([128, b], BF16, tag="Qb")
                nc.vector.tensor_mul(qa[:], rca[:], vtile[:])
                nc.vector.tensor_mul(qb[:], rcb[:], vtile[:])
                ma = sb.tile([128, b], BF16, tag="Ma")
                mb = sb.tile([128, b], BF16, tag="Mb")
                nc.gpsimd.tensor_mul(ma[:], sa[:], pa[:])
                nc.gpsimd.tensor_mul(mb[:], sbt[:], pb[:])

                # ---- dh1 = W2ra^T Qa + W2rb^T Qb - W2ra^T Ma - W2rb^T Mb ----
                dh1 = ps.tile([128, b], F32, tag="dh1")
                nc.tensor.matmul(dh1[:], c_w2pa[:], qa[:], start=True, stop=False)
                nc.tensor.matmul(dh1[:], c_w2pb[:], qb[:], start=False, stop=False)
                nc.tensor.matmul(dh1[:], c_w2ma[:], ma[:], start=False, stop=False)
                nc.tensor.matmul(dh1[:], c_w2mb[:], mb[:], start=False, stop=True)

                # ---- dA1m = (h1^2 - 1)*dh1 ; dx = -W1^T dA1m ----
                h1sq = sb.tile([128, b], BF16, tag="H1sq")
                nc.scalar.activation(h1sq[:], h1[:], AF.Square)
                da1 = sb.tile([128, b], BF16, tag="dA1")
                nc.vector.scalar_tensor_tensor(
                    da1[:], h1sq[:], 1.0, dh1[:], ALU.subtract, ALU.mult)
                nc.tensor.matmul(outq[0:64, :], c_w1n[:], da1[:],
                                 start=True, stop=True)

                # ---- PSUM -> SBUF bf16, contiguous store ----
                outs = sb.tile([128, b], BF16, tag="OUTS")
                nc.vector.tensor_copy(outs[:], outq[:])
                nc.sync.dma_start(outp[:, bass.ts(t, b)], outs[:])

    nc.finalize()
    return nc


def make_consts(W1, b1, W2, b2):
    """Host-side constant preparation (permutes W2 rows, folds signs)."""
    bf = ml_dtypes.bfloat16
    W1 = np.asarray(W1, np.float32)
    b1 = np.asarray(b1, np.float32)
    W2 = np.asarray(W2, np.float32)
    b2 = np.asarray(b2, np.float32)
    perm = np.empty(RANK * D, np.int64)
    for j in range(RANK):
        for i in range(D):
            perm[j * D + i] = i * RANK + j
    W2r = W2[perm, :]
    b2r = b2[perm]
    mblk = np.zeros((128, 128), np.float32)
    mblk[:64, :64] = 2.0
    mblk[64:, 64:] = 2.0
    msum = np.zeros((128, 64), np.float32)
    for i in range(64):
        msum[i, i] = -1.0
        msum[64 + i, i] = -1.0
    return {
        "w1t": np.ascontiguousarray(W1.T).astype(bf),
        "w2ta": np.ascontiguousarray(W2r[:128].T).astype(bf),
        "w2tb": np.ascontiguousarray(W2r[128:].T).astype(bf),
        "w2pa": np.ascontiguousarray(W2r[:128]).astype(bf),
        "w2pb": np.ascontiguousarray(W2r[128:]).astype(bf),
        "w2ma": np.ascontiguousarray(-W2r[:128]).astype(bf),
        "w2mb": np.ascontiguousarray(-W2r[128:]).astype(bf),
        "w1n": np.ascontiguousarray(-W1).astype(bf),
        "mblk": mblk.astype(bf),
        "msum": msum.astype(bf),
        "b1": b1.reshape(128, 1).astype(np.float32),
        "b2a": b2r[:128].reshape(128, 1).astype(np.float32),
        "b2b": b2r[128:].reshape(128, 1).astype(np.float32),
    }


_NC_CACHE = {}


def _get_program(n_rows, b):
    key = (n_rows, b)
    if key not in _NC_CACHE:
        _NC_CACHE[key] = build_program(n_rows, b)
    return _NC_CACHE[key]


def make_in_maps(inputs):
    input_ = np.asarray(inputs["input_"], np.float32)
    n = input_.shape[0]
    n_rows = n // N_CORES
    consts = make_consts(inputs["W1"], inputs["b1"], inputs["W2"], inputs["b2"])
    bfl = ml_dtypes.bfloat16
    in_maps = []
    for c in range(N_CORES):
        sh = input_[c * n_rows:(c + 1) * n_rows]          # [n_rows, 128]
        m = {"inp": np.ascontiguousarray(sh.T).astype(bfl)}  # [128, n_rows]
        m.update(consts)
        in_maps.append(m)
    return in_maps


def kernel(t, input_, W1, b1, W2, b2):
    input_ = np.asarray(input_, np.float32)
    n = input_.shape[0]
    n_rows = n // N_CORES
    nc = _get_program(n_rows, B)
    in_maps = make_in_maps(
        {"input_": input_, "W1": W1, "b1": b1, "W2": W2, "b2": b2})
    res = run_bass_kernel_spmd(nc, in_maps, list(range(N_CORES)))
    out = np.concatenate(
        [np.asarray(res.results[c]["out"]).astype(np.float32).T
         for c in range(N_CORES)], axis=0)
    return out



# revision 2
# speedup vs baseline: 1.0685x; 1.0685x over previous
# Trainium2 Bass kernel for nn_Connection_geognn_17076789969601.
#
# Math (per sample row of input_ [N, 128], x = row[:64], v = row[64:]):
#   h  = tanh(W1 @ x + b1)                  # [128]
#   Wm = tanh(W2 @ h + b2).reshape(64, 4)   # [64, 4]
#   u  = v @ Wm ;  H = sum(u^2)
#   output = [dH/dx, -dH/dv]
#
# Backward (per sample), with T = tanh(A2) in W2-row-permuted order so that
# column j of Wm occupies rows [64j, 64j+64):
#   dv_out = -2 Wm u
#   dA2    = 2 (v x u) * (1 - T^2) = Q - M,  Q = 2u*v,  M = 2u*v*T^2
#   dh     = W2r^T dA2 ;  dA1 = (1-h^2)*dh ;  dx = W1^T dA1
#
# v2 design (from trace analysis of v1: DVE 85% busy was the bottleneck;
# GpSimd TT ops poisoned DVE via the shared SBUF port, 416->1370ns):
#   - No GpSimd elementwise at all.
#   - Groups of G=4 subtiles (B=512 each): the pure-SBUF products P/S/Q/M
#     are single wide [128, 4096] DVE TTs (2x_1P mode), amortizing the
#     ~60-120 cycle fixed cost and slashing semaphore counts.
#   - v replicated on HOST (vrep dram tensor) - no SBUF->SBUF DMAs.
#   - Rc copy reads a 2-bank PSUM pair [128, 1024] in one Act op.
#   - out copy alternates Act/DVE to balance engine load.
#   - Engine split: Act {tanh x3, Rc, half out-copies}; DVE {P,S,Q,M wide,
#     h1sq wide, dA1 STT, half out-copies}; PE {12 matmuls}.
#
# Sharding: pure data parallel over 8 NeuronCores, batch 262144 -> 8 x 32768,
# weights replicated.

import sys

sys.path.insert(0, "/opt/trn_rl_repo")

import numpy as np
import ml_dtypes

import concourse.bass as bass
import concourse.bacc as bacc
import concourse.tile as tile
import concourse.mybir as mybir
from concourse.bass_utils import run_bass_kernel_spmd

F32 = mybir.dt.float32
BF16 = mybir.dt.bfloat16
AF = mybir.ActivationFunctionType
ALU = mybir.AluOpType

D = 64
RANK = 4
N_TOTAL = 262144
N_CORES = 8
N_ROWS = N_TOTAL // N_CORES  # 32768 per core
B = 512                      # samples per subtile (PSUM-bank sized)
G = 4                        # subtiles per wide group


def build_program(n_rows=N_ROWS, b=B, g=G):
    ng = n_rows // (b * g)   # wide groups
    gb = g * b               # samples per group (2048)
    nc = bacc.Bacc()

    xtp = nc.declare_dram_parameter("xt", [64, n_rows], BF16, isOutput=False)
    vrp = nc.declare_dram_parameter("vr", [128, n_rows], BF16, isOutput=False)
    w1t = nc.declare_dram_parameter("w1t", [64, 128], BF16, isOutput=False)
    w2ta = nc.declare_dram_parameter("w2ta", [128, 128], BF16, isOutput=False)
    w2tb = nc.declare_dram_parameter("w2tb", [128, 128], BF16, isOutput=False)
    w2pa = nc.declare_dram_parameter("w2pa", [128, 128], BF16, isOutput=False)
    w2pb = nc.declare_dram_parameter("w2pb", [128, 128], BF16, isOutput=False)
    w2ma = nc.declare_dram_parameter("w2ma", [128, 128], BF16, isOutput=False)
    w2mb = nc.declare_dram_parameter("w2mb", [128, 128], BF16, isOutput=False)
    w1n = nc.declare_dram_parameter("w1n", [128, 64], BF16, isOutput=False)
    mblk = nc.declare_dram_parameter("mblk", [128, 128], BF16, isOutput=False)
    msum = nc.declare_dram_parameter("msum", [128, 64], BF16, isOutput=False)
    b1p = nc.declare_dram_parameter("b1", [128, 1], F32, isOutput=False)
    b2ap = nc.declare_dram_parameter("b2a", [128, 1], F32, isOutput=False)
    b2bp = nc.declare_dram_parameter("b2b", [128, 1], F32, isOutput=False)
    outp = nc.declare_dram_parameter("out", [128, n_rows], BF16, isOutput=True)

    with tile.TileContext(nc) as tc:
        with (
            tc.tile_pool(name="const", bufs=1) as cp,
            tc.tile_pool(name="sb", bufs=2) as sb,
            tc.tile_pool(name="ps", bufs=1, space="PSUM") as ps,
        ):
            c_w1t = cp.tile([64, 128], BF16, tag="w1t")
            c_w2ta = cp.tile([128, 128], BF16, tag="w2ta")
            c_w2tb = cp.tile([128, 128], BF16, tag="w2tb")
            c_w2pa = cp.tile([128, 128], BF16, tag="w2pa")
            c_w2pb = cp.tile([128, 128], BF16, tag="w2pb")
            c_w2ma = cp.tile([128, 128], BF16, tag="w2ma")
            c_w2mb = cp.tile([128, 128], BF16, tag="w2mb")
            c_w1n = cp.tile([128, 64], BF16, tag="w1n")
            c_mblk = cp.tile([128, 128], BF16, tag="mblk")
            c_msum = cp.tile([128, 64], BF16, tag="msum")
            c_b1 = cp.tile([128, 1], F32, tag="b1")
            c_b2a = cp.tile([128, 1], F32, tag="b2a")
            c_b2b = cp.tile([128, 1], F32, tag="b2b")
            for t_, p_ in (
                (c_w1t, w1t), (c_w2ta, w2ta), (c_w2tb, w2tb),
                (c_w2pa, w2pa), (c_w2pb, w2pb), (c_w2ma, w2ma),
                (c_w2mb, w2mb), (c_w1n, w1n), (c_mblk, mblk),
                (c_msum, msum), (c_b1, b1p), (c_b2a, b2ap), (c_b2b, b2bp),
            ):
                nc.sync.dma_start(t_[:], p_[:])

            for gi in range(ng):
                # ---- group input loads (contiguous HBM->SBUF) ----
                xt = sb.tile([64, gb], BF16, tag="XT")
                vr = sb.tile([128, gb], BF16, tag="VR")
                nc.sync.dma_start(xt[:], xtp[:, bass.ts(gi, gb)])
                nc.sync.dma_start(vr[:], vrp[:, bass.ts(gi, gb)])

                h = sb.tile([128, gb], BF16, tag="H")
                t2 = sb.tile([128, 2 * gb], BF16, tag="T2")

                # ---- forward per subtile: a1 -> h -> a2 -> T2 ----
                for s in range(g):
                    sl = bass.ts(s, b)           # subtile cols in h/x space
                    a1 = ps.tile([128, b], F32, tag="a1")
                    nc.tensor.matmul(a1[:], c_w1t[:], xt[:, sl],
                                     start=True, stop=True)
                    nc.scalar.activation(h[:, sl], a1[:], AF.Tanh,
                                         bias=c_b1[:, 0:1])
                    a2 = ps.tile([128, 2 * b], F32, tag="a2")
                    nc.tensor.matmul(a2[:, 0:b], c_w2ta[:], h[:, sl],
                                     start=True, stop=True)
                    nc.tensor.matmul(a2[:, b:2 * b], c_w2tb[:], h[:, sl],
                                     start=True, stop=True)
                    nc.scalar.activation(t2[:, 2 * s * b:(2 * s + 1) * b],
                                         a2[:, 0:b], AF.Tanh,
                                         bias=c_b2a[:, 0:1])
                    nc.scalar.activation(t2[:, (2 * s + 1) * b:(2 * s + 2) * b],
                                         a2[:, b:2 * b], AF.Tanh,
                                         bias=c_b2b[:, 0:1])

                # ---- wide P = T2 * vrep (one DVE op over the group) ----
                pt = sb.tile([128, 2 * gb], BF16, tag="P")
                t2v = t2[:].rearrange("p (s h c) -> p s h c", s=g, h=2)
                pv = pt[:].rearrange("p (s h c) -> p s h c", s=g, h=2)
                vex = vr[:].rearrange("p (s c) -> p s c", s=g) \
                    .unsqueeze(2).broadcast_to((128, g, 2, b))
                nc.vector.tensor_mul(pv, t2v, vex)

                # ---- R = mblk @ P (2-bank PSUM pair), Rc copy (1 Act op) --
                rc = sb.tile([128, 2 * gb], BF16, tag="RC")
                for s in range(g):
                    r2 = ps.tile([128, 2 * b], F32, tag="r2")
                    nc.tensor.matmul(r2[:, 0:b], c_mblk[:],
                                     pt[:, 2 * s * b:(2 * s + 1) * b],
                                     start=True, stop=True)
                    nc.tensor.matmul(r2[:, b:2 * b], c_mblk[:],
                                     pt[:, (2 * s + 1) * b:(2 * s + 2) * b],
                                     start=True, stop=True)
                    nc.scalar.copy(rc[:, 2 * s * b:(2 * s + 2) * b], r2[:])

                # ---- wide S, Q, M products (DVE, 2x_1P) ----
                st = sb.tile([128, 2 * gb], BF16, tag="S")
                nc.vector.tensor_mul(st[:], rc[:], t2[:])
                qt = sb.tile([128, 2 * gb], BF16, tag="Q")
                qv = qt[:].rearrange("p (s h c) -> p s h c", s=g, h=2)
                rcv = rc[:].rearrange("p (s h c) -> p s h c", s=g, h=2)
                nc.vector.tensor_mul(qv, rcv, vex)
                mt = sb.tile([128, 2 * gb], BF16, tag="M")
                nc.vector.tensor_mul(mt[:], st[:], pt[:])

                # ---- wide h1sq = h * h (DVE) ----
                h1sq = sb.tile([128, gb], BF16, tag="H1SQ")
                nc.vector.tensor_mul(h1sq[:], h[:], h[:])

                # ---- backward per subtile ----
                da1 = sb.tile([128, gb], BF16, tag="DA1")
                outs = sb.tile([128, gb], BF16, tag="OUTS")
                for s in range(g):
                    sl = bass.ts(s, b)
                    sa = slice(2 * s * b, (2 * s + 1) * b)
                    sbb = slice((2 * s + 1) * b, (2 * s + 2) * b)
                    dh1 = ps.tile([128, b], F32, tag="dh1")
                    nc.tensor.matmul(dh1[:], c_w2pa[:], qt[:, sa],
                                     start=True, stop=False)
                    nc.tensor.matmul(dh1[:], c_w2pb[:], qt[:, sbb],
                                     start=False, stop=False)
                    nc.tensor.matmul(dh1[:], c_w2ma[:], mt[:, sa],
                                     start=False, stop=False)
                    nc.tensor.matmul(dh1[:], c_w2mb[:], mt[:, sbb],
                                     start=False, stop=True)
                    nc.vector.scalar_tensor_tensor(
                        da1[:, sl], h1sq[:, sl], 1.0, dh1[:],
                        ALU.subtract, ALU.mult)
                    outq = ps.tile([128, b], F32, tag="outq", bufs=2)
                    nc.tensor.matmul(outq[0:64, :], c_w1n[:], da1[:, sl],
                                     start=True, stop=True)
                    nc.tensor.matmul(outq[64:128, :], c_msum[:], st[:, sa],
                                     start=True, stop=False)
                    nc.tensor.matmul(outq[64:128, :], c_msum[:], st[:, sbb],
                                     start=False, stop=True)
                    if s % 2 == 0:
                        nc.scalar.copy(outs[:, sl], outq[:])
                    else:
                        nc.vector.tensor_copy(outs[:, sl], outq[:])

                nc.sync.dma_start(outp[:, bass.ts(gi, gb)], outs[:])

    nc.finalize()
    return nc


def make_consts(W1, b1, W2, b2):
    """Host-side constant preparation (permutes W2 rows, folds signs)."""
    bf = ml_dtypes.bfloat16
    W1 = np.asarray(W1, np.float32)
    b1 = np.asarray(b1, np.float32)
    W2 = np.asarray(W2, np.float32)
    b2 = np.asarray(b2, np.float32)
    perm = np.empty(RANK * D, np.int64)
    for j in range(RANK):
        for i in range(D):
            perm[j * D + i] = i * RANK + j
    W2r = W2[perm, :]
    b2r = b2[perm]
    mblk = np.zeros((128, 128), np.float32)
    mblk[:64, :64] = 2.0
    mblk[64:, 64:] = 2.0
    msum = np.zeros((128, 64), np.float32)
    for i in range(64):
        msum[i, i] = -1.0
        msum[64 + i, i] = -1.0
    return {
        "w1t": np.ascontiguousarray(W1.T).astype(bf),
        "w2ta": np.ascontiguousarray(W2r[:128].T).astype(bf),
        "w2tb": np.ascontiguousarray(W2r[128:].T).astype(bf),
        "w2pa": np.ascontiguousarray(W2r[:128]).astype(bf),
        "w2pb": np.ascontiguousarray(W2r[128:]).astype(bf),
        "w2ma": np.ascontiguousarray(-W2r[:128]).astype(bf),
        "w2mb": np.ascontiguousarray(-W2r[128:]).astype(bf),
        "w1n": np.ascontiguousarray(-W1).astype(bf),
        "mblk": mblk.astype(bf),
        "msum": msum.astype(bf),
        "b1": b1.reshape(128, 1).astype(np.float32),
        "b2a": b2r[:128].reshape(128, 1).astype(np.float32),
        "b2b": b2r[128:].reshape(128, 1).astype(np.float32),
    }


_NC_CACHE = {}


def _get_program(n_rows, b):
    key = (n_rows, b)
    if key not in _NC_CACHE:
        _NC_CACHE[key] = build_program(n_rows, b)
    return _NC_CACHE[key]


def make_in_maps(inputs):
    input_ = np.asarray(inputs["input_"], np.float32)
    n = input_.shape[0]
    n_rows = n // N_CORES
    consts = make_consts(inputs["W1"], inputs["b1"], inputs["W2"], inputs["b2"])
    bfl = ml_dtypes.bfloat16
    in_maps = []
    for c in range(N_CORES):
        sh = input_[c * n_rows:(c + 1) * n_rows]          # [n_rows, 128]
        xt = np.ascontiguousarray(sh[:, :64].T).astype(bfl)    # [64, n_rows]
        vt = np.ascontiguousarray(sh[:, 64:].T).astype(bfl)    # [64, n_rows]
        vr = np.concatenate([vt, vt], axis=0)                  # [128, n_rows]
        m = {"xt": xt, "vr": np.ascontiguousarray(vr)}
        m.update(consts)
        in_maps.append(m)
    return in_maps


def kernel(t, input_, W1, b1, W2, b2):
    input_ = np.asarray(input_, np.float32)
    n = input_.shape[0]
    n_rows = n // N_CORES
    nc = _get_program(n_rows, B)
    in_maps = make_in_maps(
        {"input_": input_, "W1": W1, "b1": b1, "W2": W2, "b2": b2})
    res = run_bass_kernel_spmd(nc, in_maps, list(range(N_CORES)))
    out = np.concatenate(
        [np.asarray(res.results[c]["out"]).astype(np.float32).T
         for c in range(N_CORES)], axis=0)
    return out
